# revision 24
# baseline (speedup 1.0000x reference)
"""Trainium2 Bass kernel for:
    S = sigmoid(x[:,None,None,:] * w - q)      # [B, OUT, M, IN]
    A = tanh(m)                                # [OUT, 1, IN]
    D = sum(S * A, axis=3)                     # [B, OUT, M]
    O = sum(sigmoid(D), axis=2)                # [B, OUT]
with B=256, OUT=256, M=8, IN=512 (fp32 inputs).

Distribution: tensor-parallel over OUT across 8 NeuronCores (32 output
neurons per core); x is replicated.  No collectives needed — each core
computes its O[:, o_shard] slice and the host concatenates.

The kernel is sigmoid-throughput-bound: 33.5M sigmoid evaluations per
core and only ScalarE evaluates them (1 elem/lane/cycle).  The affine
t = w*x - q is therefore spread over three engines to keep every
engine near its floor (i = IN index on partitions, 4 tiles of 128;
oms = (out_neuron, m) pairs, 256 per core, in groups of 32):

  path 1 (28 oms/group, DVE): fused tensor_scalar t = (x*w) + (-q)
        with per-partition fp32 scalars, bf16 streams; one big
        [128, 28*256] ACT sigmoid per group.
  path 2 (N_AF oms/group, ACT): fused sigmoid(scale*x + bias) with
        per-partition scale=w, bias=-q (no DVE work; off by default —
        the per-instruction fixed cost outweighed the DVE relief).
  path 3 (4 oms/group, PE):  t = diag(w) @ xT  accumulated with a
        rank-2 matmul (-q rows x ones-selector) in PSUM; ACT drains
        sigmoid(PSUM) -> SBUF in one [128, 1024] instruction.

  reduction (PE): D[o, mm, b] += A[o, i] . S[i, om, b] via matmuls
        whose stationary weights are zero-padded [128, 32] tiles with
        tanh(m) in column o_local — places each output row at its
        PSUM partition while adding zero elsewhere.
  epilogue: sigmoid(D) on the [32, 2048] PSUM accumulator (ACT),
        reduce over mm (DVE, strided view) -> O^T shard [32, 256].
"""

import sys

if "/opt/trn_rl_repo" not in sys.path:
    sys.path.insert(0, "/opt/trn_rl_repo")

import numpy as np


def _install_profile_shims():
    """If this environment lacks antenv.axon_hooks (run_bass_kernel_spmd
    imports it on the trace=True path), register a working ctypes-based
    NTFF hook so tracing degrades gracefully instead of crashing, and
    make upload_artifacts failure non-fatal."""
    try:
        from antenv import axon_hooks  # noqa: F401
        return
    except ImportError:
        pass
    import contextlib
    import ctypes
    import types

    def _hook_factory():
        try:
            lib = ctypes.CDLL("/opt/axon/libaxon_pjrt.so")
            if not hasattr(lib, "axon_start_nrt_profile"):
                return None
        except OSError:
            return None
        lib.axon_start_nrt_profile.argtypes = [
            ctypes.POINTER(ctypes.c_int64),
            ctypes.c_size_t,
        ]
        lib.axon_start_nrt_profile.restype = ctypes.c_int64
        lib.axon_stop_nrt_profile.argtypes = [ctypes.c_char_p]
        lib.axon_stop_nrt_profile.restype = ctypes.c_int64

        @contextlib.contextmanager
        def _hook(output_dir, device_ids):
            import jax

            jax.devices()
            if device_ids:
                ids = (ctypes.c_int64 * len(device_ids))(*device_ids)
                rc = lib.axon_start_nrt_profile(ids, len(device_ids))
            else:
                rc = lib.axon_start_nrt_profile(None, 0)
            if rc != 0:
                raise RuntimeError(f"axon_start_nrt_profile rc={rc}")
            try:
                yield
            finally:
                lib.axon_stop_nrt_profile(str(output_dir).encode())

        return _hook

    mod = types.ModuleType("antenv.axon_hooks")
    mod.get_axon_ntff_profile_hook = _hook_factory
    mod.set_axon_ntff_profile_hook = lambda h: None
    sys.modules["antenv.axon_hooks"] = mod

    from concourse import bass_utils as _bu

    _orig_upload = _bu.upload_artifacts

    def _safe_upload(tmpdir):
        try:
            return _orig_upload(tmpdir)
        except Exception:
            return f"local://{tmpdir}"

    _bu.upload_artifacts = _safe_upload


_install_profile_shims()

B, OUT, M, IN = 256, 256, 8, 512
NCORES = 8
O_PER_CORE = OUT // NCORES          # 32
OM_PER_CORE = O_PER_CORE * M        # 256
NIT = IN // 128                     # 4 partition tiles over IN
OM_BLK = 32                         # oms per (it, blk) group
NBLK = OM_PER_CORE // OM_BLK        # 8
N_PE = 4                            # oms per group on the PE-affine path
                                    # (4 om PSUM slots x 2 bufs + the D
                                    # accumulator exactly fill 8 banks)
N_AF = 0                            # oms per group fully on ACT
N_DVE = OM_BLK - N_PE - N_AF        # 27

_CACHE = {}


def _build_nc():
    import concourse.bacc as bacc
    import concourse.bass as bass
    import concourse.mybir as mybir
    import concourse.tile as tile

    f32 = mybir.dt.float32
    bf16 = mybir.dt.bfloat16
    Act = mybir.ActivationFunctionType
    Alu = mybir.AluOpType

    nc = bacc.Bacc("TRN2", target_bir_lowering=False, debug=False)

    xT_d = nc.dram_tensor("xT", [128, NIT, B], bf16, kind="ExternalInput")
    wT_d = nc.dram_tensor("wT", [128, NIT, OM_PER_CORE], f32, kind="ExternalInput")
    # staged NEGATED: the addend/bias is -q on every path
    qT_d = nc.dram_tensor("qT", [128, NIT, OM_PER_CORE], f32, kind="ExternalInput")
    mT_d = nc.dram_tensor("mT", [128, NIT * O_PER_CORE], f32, kind="ExternalInput")
    # host-built diag(w) stationary tiles for the PE-affine path:
    # one [128, 128] bf16 diag per (it, blk, k<N_PE)
    nd = NIT * NBLK * N_PE
    wdiag_d = nc.dram_tensor("wdiag", [128, nd, 128], bf16, kind="ExternalInput")
    # host-built -q rows for the rank-2 bias matmul: [2, (it, blk, pair), 128]
    npair = NIT * NBLK * (N_PE // 2)
    qpe_d = nc.dram_tensor("qpe", [2, npair, 128], bf16, kind="ExternalInput")
    ones2_d = nc.dram_tensor("ones2", [2, 2, B], bf16, kind="ExternalInput")
    out_d = nc.dram_tensor("out", [O_PER_CORE, B], f32, kind="ExternalOutput")

    with tile.TileContext(nc) as tc:
        with (
            tc.tile_pool(name="consts", bufs=1) as consts,
            tc.tile_pool(name="tpool", bufs=3) as tpool,
            tc.tile_pool(name="spool", bufs=3) as spool,
            tc.tile_pool(name="psum", bufs=1, space="PSUM") as psum,
            tc.tile_pool(name="psum2", bufs=2, space="PSUM") as psum2,
            tc.tile_pool(name="epi", bufs=1) as epi,
        ):
            xT = consts.tile([128, NIT, B], bf16)
            wT = consts.tile([128, NIT, OM_PER_CORE], f32)
            qT = consts.tile([128, NIT, OM_PER_CORE], f32)
            mT = consts.tile([128, NIT * O_PER_CORE], f32)
            wdiag = consts.tile([128, nd, 128], bf16)
            qpe = consts.tile([2, npair, 128], bf16)
            ones2 = consts.tile([2, 2, B], bf16)
            a16 = consts.tile([128, NIT * O_PER_CORE], bf16)
            # zero-padded stationary weights: block (it, o) holds tanh(m)
            # for (o, i-tile it) in column o, zeros elsewhere
            apad = consts.tile([128, NIT * O_PER_CORE, O_PER_CORE], bf16)

            for it in range(NIT):
                nc.sync.dma_start(out=xT[:, it, :], in_=xT_d.ap()[:, it, :])
                nc.sync.dma_start(out=wT[:, it, :], in_=wT_d.ap()[:, it, :])
                nc.sync.dma_start(out=qT[:, it, :], in_=qT_d.ap()[:, it, :])
            nc.sync.dma_start(out=mT, in_=mT_d.ap())
            nq = NBLK * N_PE
            for it in range(NIT):
                nc.sync.dma_start(
                    out=wdiag[:, it * nq : (it + 1) * nq, :],
                    in_=wdiag_d.ap()[:, it * nq : (it + 1) * nq, :],
                )
            nc.sync.dma_start(out=qpe, in_=qpe_d.ap())
            # rank-2 selector: row k is ones over b for pair-half k
            nc.sync.dma_start(out=ones2, in_=ones2_d.ap())

            nc.scalar.activation(a16, mT, Act.Tanh)
            apad_flat = apad.rearrange("p a b -> p (a b)")
            nc.gpsimd.memset(apad_flat, 0.0)

            def build_apad():
                # emitted after group 0's DVE ops: keeps the diag copies
                # (which wait on mT DMA -> tanh) off the head of DVE's
                # in-order stream; apad is first read at pipeline step 1
                blk_w = O_PER_CORE  # 32 columns per (it, o) block
                for it in range(NIT):
                    # diagonal strided view: col (it*32+o)*32 + o, o in 0..31
                    base = apad_flat[:, it * blk_w * blk_w : (it + 1) * blk_w * blk_w]
                    diag = bass.AP(
                        tensor=base.tensor,
                        offset=base.offset,
                        ap=[base.ap[0], [blk_w + 1, blk_w]],
                    )
                    nc.vector.tensor_copy(diag, a16[:, it * blk_w : (it + 1) * blk_w])

            dps = psum.tile([O_PER_CORE, M * B], f32)

            j_pe = N_DVE + N_AF
            h1 = N_DVE // 2

            def emit_dve(it, blk, s):
                t = tpool.tile([128, N_DVE, B], bf16, tag="t")
                for j in range(N_DVE):
                    om = blk * OM_BLK + j
                    nc.vector.tensor_scalar(
                        t[:, j, :],
                        xT[:, it, :],
                        wT[:, it, om : om + 1],
                        qT[:, it, om : om + 1],
                        Alu.mult,
                        Alu.add,
                    )
                nc.scalar.activation(s[:, :N_DVE, :], t, Act.Sigmoid)
                for j in range(N_DVE, N_DVE + N_AF):
                    om = blk * OM_BLK + j
                    nc.scalar.activation(
                        s[:, j, :],
                        xT[:, it, :],
                        Act.Sigmoid,
                        bias=qT[:, it, om : om + 1],
                        scale=wT[:, it, om : om + 1],
                    )

            def emit_affine_pe(it, blk):
                gi = it * NBLK + blk
                tps = psum2.tile([128, N_PE, B], f32, tag="tps")
                for k in range(N_PE):
                    # start=True zeroes a whole 2KB PSUM bank (2 om slots),
                    # so only the first write per bank sets it
                    nc.tensor.matmul(
                        tps[:, k, :],
                        wdiag[:, gi * N_PE + k, :],
                        xT[:, it, :],
                        start=(k % 2 == 0),
                        stop=False,
                        skip_group_check=True,
                    )
                for pr in range(N_PE // 2):
                    nc.tensor.matmul(
                        tps[:, 2 * pr : 2 * pr + 2, :],
                        qpe[:, gi * (N_PE // 2) + pr, :],
                        ones2.rearrange("p a b -> p (a b)"),
                        start=False,
                        stop=True,
                        skip_group_check=True,
                    )
                return tps

            def emit_drain(s, tps):
                nc.scalar.activation(s[:, j_pe : j_pe + N_PE, :], tps, Act.Sigmoid)

            def emit_reduction(it, blk, s):
                for o4 in range(OM_BLK // M):
                    o_loc = blk * (OM_BLK // M) + o4
                    lhsT = apad[:, it * O_PER_CORE + o_loc, :]
                    for p4 in range(4):
                        rhs = s[:, o4 * M + 2 * p4 : o4 * M + 2 * p4 + 2, :]
                        outp = dps[:, p4 * 512 : (p4 + 1) * 512]
                        first = it == 0 and blk == 0 and o4 == 0
                        last = it == NIT - 1 and blk == NBLK - 1 and o4 == 3
                        nc.tensor.matmul(
                            outp,
                            lhsT,
                            rhs,
                            start=first,
                            stop=last,
                            skip_group_check=True,
                        )

            # one-group software pipeline: PE-affine for group g runs while
            # PE-reduction consumes group g-1; ACT drains g-1's PSUM first
            prev = None
            for it in range(NIT):
                for blk in range(NBLK):
                    s = spool.tile([128, OM_BLK, B], bf16)
                    if prev is not None:
                        emit_drain(prev[2], prev[3])
                    emit_dve(it, blk, s)
                    if prev is None:
                        build_apad()
                    tps = emit_affine_pe(it, blk)
                    if prev is not None:
                        emit_reduction(prev[0], prev[1], prev[2])
                    prev = (it, blk, s, tps)
            emit_drain(prev[2], prev[3])
            emit_reduction(prev[0], prev[1], prev[2])

            dsig = epi.tile([O_PER_CORE, M * B], f32)
            nc.scalar.activation(dsig, dps, Act.Sigmoid)
            osb = epi.tile([O_PER_CORE, B], f32)
            # sum over mm: view [32, b, mm] with mm innermost (stride 256)
            dv = dsig.rearrange("p (mm b) -> p b mm", mm=M)
            nc.vector.tensor_reduce(osb, dv, mybir.AxisListType.X, Alu.add)
            nc.sync.dma_start(out=out_d.ap(), in_=osb)

    nc.compile()
    return nc


def _get_nc():
    if "nc" not in _CACHE:
        _CACHE["nc"] = _build_nc()
    return _CACHE["nc"]


def _prep_in_maps(x, w, q, m):
    import ml_dtypes

    x = np.asarray(x, np.float32)
    w = np.asarray(w, np.float32)
    q = np.asarray(q, np.float32)
    m = np.asarray(m, np.float32)

    # x^T tiled: xT[p, it, b] = x[b, it*128+p]
    xt = np.ascontiguousarray(
        x.T.reshape(NIT, 128, B).transpose(1, 0, 2)
    ).astype(ml_dtypes.bfloat16)

    j_pe = N_DVE + N_AF
    nd = NIT * NBLK * N_PE
    npair = NIT * NBLK * (N_PE // 2)
    ii = np.arange(128)
    in_maps = []
    for c in range(NCORES):
        o0 = c * O_PER_CORE
        ws = w[o0 : o0 + O_PER_CORE].reshape(OM_PER_CORE, IN)
        qs = -q[o0 : o0 + O_PER_CORE].reshape(OM_PER_CORE, IN)
        ms = m[o0 : o0 + O_PER_CORE, 0, :]  # [32, 512]
        wt = np.ascontiguousarray(ws.T.reshape(NIT, 128, OM_PER_CORE).transpose(1, 0, 2))
        qt = np.ascontiguousarray(qs.T.reshape(NIT, 128, OM_PER_CORE).transpose(1, 0, 2))
        mt = np.ascontiguousarray(
            ms.T.reshape(NIT, 128, O_PER_CORE).transpose(1, 0, 2)
        ).reshape(128, NIT * O_PER_CORE)
        # PE-affine stationary tiles: diag(w[om, it_slice]) per (it, blk, k)
        wdiag = np.zeros((128, nd, 128), np.float32)
        qpe = np.zeros((2, npair, 128), np.float32)
        for it in range(NIT):
            for blk in range(NBLK):
                gi = it * NBLK + blk
                for k in range(N_PE):
                    om = blk * OM_BLK + j_pe + k
                    wdiag[ii, gi * N_PE + k, ii] = ws[om, it * 128 : (it + 1) * 128]
                for pr in range(N_PE // 2):
                    om0 = blk * OM_BLK + j_pe + 2 * pr
                    qpe[0, gi * (N_PE // 2) + pr, :] = qs[om0, it * 128 : (it + 1) * 128]
                    qpe[1, gi * (N_PE // 2) + pr, :] = qs[
                        om0 + 1, it * 128 : (it + 1) * 128
                    ]
        sel = np.zeros((2, 2, B), np.float32)
        sel[0, 0, :] = 1.0
        sel[1, 1, :] = 1.0
        in_maps.append(
            {
                "ones2": sel.astype(ml_dtypes.bfloat16),
                "xT": xt,
                "wT": wt,
                "qT": qt,
                "mT": mt,
                "wdiag": wdiag.astype(ml_dtypes.bfloat16),
                "qpe": qpe.astype(ml_dtypes.bfloat16),
            }
        )
    return in_maps


def kernel(x, w, q, m):
    from concourse import bass_utils

    nc = _get_nc()
    in_maps = _prep_in_maps(x, w, q, m)
    res = bass_utils.run_bass_kernel_spmd(
        nc, in_maps, core_ids=list(range(NCORES)), trace=False
    )
    parts = [res.results[c]["out"] for c in range(NCORES)]  # each [32, 256] = O^T shard
    return np.ascontiguousarray(np.concatenate(parts, axis=0).T.astype(np.float32))


# revision 25
# speedup vs baseline: 1.2083x; 1.2083x over previous
"""Trainium2 Bass kernel for:
    S = sigmoid(x[:,None,None,:] * w - q)      # [B, OUT, M, IN]
    A = tanh(m)                                # [OUT, 1, IN]
    D = sum(S * A, axis=3)                     # [B, OUT, M]
    O = sum(sigmoid(D), axis=2)                # [B, OUT]
with B=256, OUT=256, M=8, IN=512 (fp32 inputs).

Distribution: tensor-parallel over OUT across 8 NeuronCores (32 output
neurons per core); x is replicated.  No collectives needed — each core
computes its O[:, o_shard] slice and the host concatenates.

The kernel is sigmoid-throughput-bound: 33.5M sigmoid evaluations per
core and only ScalarE evaluates them (1 elem/lane/cycle).  The affine
t = w*x - q is therefore spread over three engines to keep every
engine near its floor (i = IN index on partitions, 4 tiles of 128;
oms = (out_neuron, m) pairs, 256 per core, in groups of 32):

  path 1 (28 oms/group, DVE): fused tensor_scalar t = (x*w) + (-q)
        with per-partition fp32 scalars, bf16 streams; one big
        [128, 28*256] ACT sigmoid per group.
  path 2 (N_AF oms/group, ACT): fused sigmoid(scale*x + bias) with
        per-partition scale=w, bias=-q (no DVE work; off by default —
        the per-instruction fixed cost outweighed the DVE relief).
  path 3 (4 oms/group, PE):  t = diag(w) @ xT  accumulated with a
        rank-2 matmul (-q rows x ones-selector) in PSUM; ACT drains
        sigmoid(PSUM) -> SBUF in one [128, 1024] instruction.

  reduction (PE): D[o, mm, b] += A[o, i] . S[i, om, b] via matmuls
        whose stationary weights are zero-padded [128, 32] tiles with
        tanh(m) in column o_local — places each output row at its
        PSUM partition while adding zero elsewhere.
  epilogue: sigmoid(D) on the [32, 2048] PSUM accumulator (ACT),
        reduce over mm (DVE, strided view) -> O^T shard [32, 256].
"""

import sys

if "/opt/trn_rl_repo" not in sys.path:
    sys.path.insert(0, "/opt/trn_rl_repo")

import numpy as np


def _install_profile_shims():
    """If this environment lacks antenv.axon_hooks (run_bass_kernel_spmd
    imports it on the trace=True path), register a working ctypes-based
    NTFF hook so tracing degrades gracefully instead of crashing, and
    make upload_artifacts failure non-fatal."""
    try:
        from antenv import axon_hooks  # noqa: F401
        return
    except ImportError:
        pass
    import contextlib
    import ctypes
    import types

    def _hook_factory():
        try:
            lib = ctypes.CDLL("/opt/axon/libaxon_pjrt.so")
            if not hasattr(lib, "axon_start_nrt_profile"):
                return None
        except OSError:
            return None
        lib.axon_start_nrt_profile.argtypes = [
            ctypes.POINTER(ctypes.c_int64),
            ctypes.c_size_t,
        ]
        lib.axon_start_nrt_profile.restype = ctypes.c_int64
        lib.axon_stop_nrt_profile.argtypes = [ctypes.c_char_p]
        lib.axon_stop_nrt_profile.restype = ctypes.c_int64

        @contextlib.contextmanager
        def _hook(output_dir, device_ids):
            import jax

            jax.devices()
            if device_ids:
                ids = (ctypes.c_int64 * len(device_ids))(*device_ids)
                rc = lib.axon_start_nrt_profile(ids, len(device_ids))
            else:
                rc = lib.axon_start_nrt_profile(None, 0)
            if rc != 0:
                raise RuntimeError(f"axon_start_nrt_profile rc={rc}")
            try:
                yield
            finally:
                lib.axon_stop_nrt_profile(str(output_dir).encode())

        return _hook

    mod = types.ModuleType("antenv.axon_hooks")
    mod.get_axon_ntff_profile_hook = _hook_factory
    mod.set_axon_ntff_profile_hook = lambda h: None
    sys.modules["antenv.axon_hooks"] = mod

    from concourse import bass_utils as _bu

    _orig_upload = _bu.upload_artifacts

    def _safe_upload(tmpdir):
        try:
            return _orig_upload(tmpdir)
        except Exception:
            return f"local://{tmpdir}"

    _bu.upload_artifacts = _safe_upload


_install_profile_shims()

B, OUT, M, IN = 256, 256, 8, 512
NCORES = 8
O_PER_CORE = OUT // NCORES          # 32
OM_PER_CORE = O_PER_CORE * M        # 256
NIT = IN // 128                     # 4 partition tiles over IN
OM_BLK = 32                         # oms per (it, blk) group
NBLK = OM_PER_CORE // OM_BLK        # 8
N_PE = 4                            # oms per group on the PE-affine path
                                    # (4 om PSUM slots x 2 bufs + the D
                                    # accumulator exactly fill 8 banks)
N_AF = 0                            # oms per group fully on ACT
N_DVE = OM_BLK - N_PE - N_AF        # 27

_CACHE = {}


def _build_nc():
    import concourse.bacc as bacc
    import concourse.bass as bass
    import concourse.mybir as mybir
    import concourse.tile as tile

    f32 = mybir.dt.float32
    bf16 = mybir.dt.bfloat16
    Act = mybir.ActivationFunctionType
    Alu = mybir.AluOpType

    nc = bacc.Bacc("TRN2", target_bir_lowering=False, debug=False)

    xT_d = nc.dram_tensor("xT", [128, NIT, B], bf16, kind="ExternalInput")
    wT_d = nc.dram_tensor("wT", [128, NIT, OM_PER_CORE], f32, kind="ExternalInput")
    # staged NEGATED: the addend/bias is -q on every path
    qT_d = nc.dram_tensor("qT", [128, NIT, OM_PER_CORE], f32, kind="ExternalInput")
    mT_d = nc.dram_tensor("mT", [128, NIT * O_PER_CORE], f32, kind="ExternalInput")
    # host-built diag(w) stationary tiles for the PE-affine path:
    # one [128, 128] bf16 diag per (it, blk, k<N_PE)
    nd = NIT * NBLK * N_PE
    wdiag_d = nc.dram_tensor("wdiag", [128, nd, 128], bf16, kind="ExternalInput")
    # host-built -q rows for the rank-2 bias matmul: [2, (it, blk, pair), 128]
    npair = NIT * NBLK * (N_PE // 2)
    qpe_d = nc.dram_tensor("qpe", [2, npair, 128], bf16, kind="ExternalInput")
    ones2_d = nc.dram_tensor("ones2", [2, 2, B], bf16, kind="ExternalInput")
    out_d = nc.dram_tensor("out", [O_PER_CORE, B], f32, kind="ExternalOutput")

    with tile.TileContext(nc) as tc:
        with (
            tc.tile_pool(name="consts", bufs=1) as consts,
            tc.tile_pool(name="tpool", bufs=3) as tpool,
            tc.tile_pool(name="spool", bufs=3) as spool,
            tc.tile_pool(name="psum", bufs=1, space="PSUM") as psum,
            tc.tile_pool(name="psum2", bufs=2, space="PSUM") as psum2,
            tc.tile_pool(name="epi", bufs=1) as epi,
        ):
            xT = consts.tile([128, NIT, B], bf16)
            wT = consts.tile([128, NIT, OM_PER_CORE], f32)
            qT = consts.tile([128, NIT, OM_PER_CORE], f32)
            mT = consts.tile([128, NIT * O_PER_CORE], f32)
            wdiag = consts.tile([128, nd, 128], bf16)
            qpe = consts.tile([2, npair, 128], bf16)
            ones2 = consts.tile([2, 2, B], bf16)
            a16 = consts.tile([128, NIT * O_PER_CORE], bf16)
            # zero-padded stationary weights: block (it, o) holds tanh(m)
            # for (o, i-tile it) in column o, zeros elsewhere
            apad = consts.tile([128, NIT * O_PER_CORE, O_PER_CORE], bf16)

            for it in range(NIT):
                nc.sync.dma_start(out=xT[:, it, :], in_=xT_d.ap()[:, it, :])
                nc.sync.dma_start(out=wT[:, it, :], in_=wT_d.ap()[:, it, :])
                nc.sync.dma_start(out=qT[:, it, :], in_=qT_d.ap()[:, it, :])
            nc.sync.dma_start(out=mT, in_=mT_d.ap())
            nq = NBLK * N_PE
            for it in range(NIT):
                nc.sync.dma_start(
                    out=wdiag[:, it * nq : (it + 1) * nq, :],
                    in_=wdiag_d.ap()[:, it * nq : (it + 1) * nq, :],
                )
            nc.sync.dma_start(out=qpe, in_=qpe_d.ap())
            # rank-2 selector: row k is ones over b for pair-half k
            nc.sync.dma_start(out=ones2, in_=ones2_d.ap())

            nc.scalar.activation(a16, mT, Act.Tanh)
            apad_flat = apad.rearrange("p a b -> p (a b)")
            nc.gpsimd.memset(apad_flat, 0.0)

            def build_apad():
                # emitted after group 0's DVE ops: keeps the diag copies
                # (which wait on mT DMA -> tanh) off the head of DVE's
                # in-order stream; apad is first read at pipeline step 1
                blk_w = O_PER_CORE  # 32 columns per (it, o) block
                for it in range(NIT):
                    # diagonal strided view: col (it*32+o)*32 + o, o in 0..31
                    base = apad_flat[:, it * blk_w * blk_w : (it + 1) * blk_w * blk_w]
                    diag = bass.AP(
                        tensor=base.tensor,
                        offset=base.offset,
                        ap=[base.ap[0], [blk_w + 1, blk_w]],
                    )
                    nc.vector.tensor_copy(diag, a16[:, it * blk_w : (it + 1) * blk_w])

            dps = psum.tile([O_PER_CORE, M * B], f32)

            j_pe = N_DVE + N_AF
            h1 = N_DVE // 2

            def emit_dve(it, blk, s):
                t = tpool.tile([128, N_DVE, B], bf16, tag="t")
                for j in range(N_DVE):
                    om = blk * OM_BLK + j
                    nc.vector.tensor_scalar(
                        t[:, j, :],
                        xT[:, it, :],
                        wT[:, it, om : om + 1],
                        qT[:, it, om : om + 1],
                        Alu.mult,
                        Alu.add,
                    )
                nc.scalar.activation(s[:, :N_DVE, :], t, Act.Sigmoid)
                for j in range(N_DVE, N_DVE + N_AF):
                    om = blk * OM_BLK + j
                    nc.scalar.activation(
                        s[:, j, :],
                        xT[:, it, :],
                        Act.Sigmoid,
                        bias=qT[:, it, om : om + 1],
                        scale=wT[:, it, om : om + 1],
                    )

            def emit_affine_pe(it, blk):
                gi = it * NBLK + blk
                tps = psum2.tile([128, N_PE, B], f32, tag="tps")
                for k in range(N_PE):
                    # start=True zeroes a whole 2KB PSUM bank (2 om slots),
                    # so only the first write per bank sets it
                    nc.tensor.matmul(
                        tps[:, k, :],
                        wdiag[:, gi * N_PE + k, :],
                        xT[:, it, :],
                        start=(k % 2 == 0),
                        stop=False,
                        skip_group_check=True,
                    )
                for pr in range(N_PE // 2):
                    nc.tensor.matmul(
                        tps[:, 2 * pr : 2 * pr + 2, :],
                        qpe[:, gi * (N_PE // 2) + pr, :],
                        ones2.rearrange("p a b -> p (a b)"),
                        start=False,
                        stop=True,
                        skip_group_check=True,
                    )
                return tps

            def emit_drain(s, tps):
                nc.scalar.activation(s[:, j_pe : j_pe + N_PE, :], tps, Act.Sigmoid)

            def emit_reduction(it, blk, s):
                for o4 in range(OM_BLK // M):
                    o_loc = blk * (OM_BLK // M) + o4
                    lhsT = apad[:, it * O_PER_CORE + o_loc, :]
                    for p4 in range(4):
                        rhs = s[:, o4 * M + 2 * p4 : o4 * M + 2 * p4 + 2, :]
                        outp = dps[:, p4 * 512 : (p4 + 1) * 512]
                        first = it == 0 and blk == 0 and o4 == 0
                        last = it == NIT - 1 and blk == NBLK - 1 and o4 == 3
                        nc.tensor.matmul(
                            outp,
                            lhsT,
                            rhs,
                            start=first,
                            stop=last,
                            skip_group_check=True,
                        )

            # one-group software pipeline: PE-affine for group g runs while
            # PE-reduction consumes group g-1; ACT drains g-1's PSUM first
            prev = None
            for it in range(NIT):
                for blk in range(NBLK):
                    s = spool.tile([128, OM_BLK, B], bf16)
                    if prev is not None:
                        emit_drain(prev[2], prev[3])
                    emit_dve(it, blk, s)
                    if prev is None:
                        build_apad()
                    tps = emit_affine_pe(it, blk)
                    if prev is not None:
                        emit_reduction(prev[0], prev[1], prev[2])
                    prev = (it, blk, s, tps)
            emit_drain(prev[2], prev[3])
            emit_reduction(prev[0], prev[1], prev[2])

            dsig = epi.tile([O_PER_CORE, M * B], bf16)
            nc.scalar.activation(dsig, dps, Act.Sigmoid)
            # sum over mm as a pairwise tree: halves are (mm, mm+4) aligned
            # elementwise, so each level is a contiguous bf16 add (DVE 2x)
            r1 = epi.tile([O_PER_CORE, M * B // 2], bf16)
            nc.vector.tensor_tensor(
                r1, dsig[:, : M * B // 2], dsig[:, M * B // 2 :], Alu.add
            )
            r2 = epi.tile([O_PER_CORE, M * B // 4], bf16)
            nc.vector.tensor_tensor(
                r2, r1[:, : M * B // 4], r1[:, M * B // 4 :], Alu.add
            )
            osb = epi.tile([O_PER_CORE, B], f32)
            nc.vector.tensor_tensor(osb, r2[:, :B], r2[:, B:], Alu.add)
            nc.sync.dma_start(out=out_d.ap(), in_=osb)

    nc.compile()
    return nc


def _get_nc():
    if "nc" not in _CACHE:
        _CACHE["nc"] = _build_nc()
    return _CACHE["nc"]


def _prep_in_maps(x, w, q, m):
    import ml_dtypes

    x = np.asarray(x, np.float32)
    w = np.asarray(w, np.float32)
    q = np.asarray(q, np.float32)
    m = np.asarray(m, np.float32)

    # x^T tiled: xT[p, it, b] = x[b, it*128+p]
    xt = np.ascontiguousarray(
        x.T.reshape(NIT, 128, B).transpose(1, 0, 2)
    ).astype(ml_dtypes.bfloat16)

    j_pe = N_DVE + N_AF
    nd = NIT * NBLK * N_PE
    npair = NIT * NBLK * (N_PE // 2)
    ii = np.arange(128)
    in_maps = []
    for c in range(NCORES):
        o0 = c * O_PER_CORE
        ws = w[o0 : o0 + O_PER_CORE].reshape(OM_PER_CORE, IN)
        qs = -q[o0 : o0 + O_PER_CORE].reshape(OM_PER_CORE, IN)
        ms = m[o0 : o0 + O_PER_CORE, 0, :]  # [32, 512]
        wt = np.ascontiguousarray(ws.T.reshape(NIT, 128, OM_PER_CORE).transpose(1, 0, 2))
        qt = np.ascontiguousarray(qs.T.reshape(NIT, 128, OM_PER_CORE).transpose(1, 0, 2))
        mt = np.ascontiguousarray(
            ms.T.reshape(NIT, 128, O_PER_CORE).transpose(1, 0, 2)
        ).reshape(128, NIT * O_PER_CORE)
        # PE-affine stationary tiles: diag(w[om, it_slice]) per (it, blk, k)
        wdiag = np.zeros((128, nd, 128), np.float32)
        qpe = np.zeros((2, npair, 128), np.float32)
        for it in range(NIT):
            for blk in range(NBLK):
                gi = it * NBLK + blk
                for k in range(N_PE):
                    om = blk * OM_BLK + j_pe + k
                    wdiag[ii, gi * N_PE + k, ii] = ws[om, it * 128 : (it + 1) * 128]
                for pr in range(N_PE // 2):
                    om0 = blk * OM_BLK + j_pe + 2 * pr
                    qpe[0, gi * (N_PE // 2) + pr, :] = qs[om0, it * 128 : (it + 1) * 128]
                    qpe[1, gi * (N_PE // 2) + pr, :] = qs[
                        om0 + 1, it * 128 : (it + 1) * 128
                    ]
        sel = np.zeros((2, 2, B), np.float32)
        sel[0, 0, :] = 1.0
        sel[1, 1, :] = 1.0
        in_maps.append(
            {
                "ones2": sel.astype(ml_dtypes.bfloat16),
                "xT": xt,
                "wT": wt,
                "qT": qt,
                "mT": mt,
                "wdiag": wdiag.astype(ml_dtypes.bfloat16),
                "qpe": qpe.astype(ml_dtypes.bfloat16),
            }
        )
    return in_maps


def kernel(x, w, q, m):
    from concourse import bass_utils

    nc = _get_nc()
    in_maps = _prep_in_maps(x, w, q, m)
    res = bass_utils.run_bass_kernel_spmd(
        nc, in_maps, core_ids=list(range(NCORES)), trace=False
    )
    parts = [res.results[c]["out"] for c in range(NCORES)]  # each [32, 256] = O^T shard
    return np.ascontiguousarray(np.concatenate(parts, axis=0).T.astype(np.float32))
